# revision 6
# baseline (speedup 1.0000x reference)
"""Trainium2 Bass kernel for nn_NestRQModel (NEST-RQ pretraining loss).

Data-parallel over valid CE rows (host-compacted), 8 cores, no collectives.
Each core returns partial (masked nll sum, corr count) + a vocab-presence
bitmap; the host combines them into the 4 scalar outputs.

Host compaction: only rows (b, t) with t in [1, L_b-1] (L = len//4) carry
loss/acc/presence.  They are packed contiguously and split across cores
(~713 rows/core -> RT=6 tiles of 128 instead of 8), cutting every engine's
work by 25% and removing the enc-shift / CE-mask complexity.

Per-core pipeline per 128-row tile (A leads B by two tiles):
  stage 0: q = stackT.T @ projA (PE, f32); LN mean folded in as
           x = q - rowsum*csum/320 (LN's rstd is a positive per-row scale,
           argmax-invariant -> dropped); PE transpose; Kahan bf16 hi/lo
           split -> xk [128,128] (exact f32-level dots at full PE rate).
  stage A: dots = xk.T @ Ek (PE bf16, 4x2048 psum chunks); DVE max8 +
           find_index8 directly on each psum chunk; small-op combine of the
           4 (max, idx) pairs -> code.  Pad rows sum all 4 chunk offsets
           (12288 > V-1) and are dropped by indirect-DMA bounds_check.
  prep:    gpsimd indirect-gather Wt[code] [128,512] f32; DVE mult + ACT
           Copy+accum -> exact ltgt; ethr = exp(ltgt+margin) (ACT).
  stage B: logits = encT.T @ W in fp8 e4m3 DoubleRow pairs (scales 1024/16,
           descale folded into the exp's free affine; margin 0.03 covers the
           fp8 noise, validated on the top-2 gap distribution); ACT Exp +
           accum -> sum-of-exp; corr count = sum relu(exp - ethr) on ACT
           (exactly 0 for correct rows), moved to DVE is_gt for the last two
           tiles where ACT is the tail bottleneck.
  finalize: batched [128,RT] column ops; overlapped presence scatters;
           partition reduction via ones-matmul.

DMA: single sync ring in strict consumption order (smalls/stackT/stack
rows -> Ek -> W h0 -> enc -> W rest); parallel rings share HBM bandwidth
fairly and delay the critical prefix, so they are not used.
"""
import os
import sys

import numpy as np

os.environ.setdefault("MYCRO_LOCAL_CACHE", "1")

try:
    import concourse.bass as bass
except ImportError:
    sys.path.insert(0, "/opt/trn_rl_repo")
    import concourse.bass as bass

import ml_dtypes
import concourse.bacc as bacc
import concourse.tile as tile
from concourse import mybir
from concourse.bass import IndirectOffsetOnAxis
from concourse.masks import make_identity
from contextlib import ExitStack

F32 = mybir.dt.float32
FP8 = mybir.dt.float8e4
BF16 = mybir.dt.bfloat16
U32 = mybir.dt.uint32
I32 = mybir.dt.int32
AF = mybir.ActivationFunctionType
ALU = mybir.AluOpType

NCORES = 8
B, T, F = 16, 2048, 80
STK, STRIDE = 4, 4
N = 512                 # frames per batch after subsampling
SF = STK * F            # 320 stacked feature dim
EDIM = 16
V = 8192
D = 512                 # encoder dim
VC = 1024               # vocab chunk (psum tile width, 2 psum banks)
NVC = V // VC           # 8
MARGIN = 0.03      # covers fp8 stage-B logit noise (~0.02)
SC_W, SC_E = 1024.0, 16.0   # fp8 pre-scales for W / encoder rows

_NC_CACHE = {}


def _build_program(RT):
    if RT in _NC_CACHE:
        return _NC_CACHE[RT]
    RC = RT * 128
    nc = bacc.Bacc("TRN2", target_bir_lowering=False)

    stack_rows = nc.declare_dram_parameter("stack_rows", [RC, SF], F32, isOutput=False)
    stackT = nc.declare_dram_parameter("stackT", [SF, RC], F32, isOutput=False)
    smalls = nc.declare_dram_parameter("smalls", [128, 4 * EDIM], F32,
                                       isOutput=False)
    ek = nc.declare_dram_parameter("Ek", [128, V], BF16, isOutput=False)
    w = nc.declare_dram_parameter("W", [2, 128, 2, V], FP8, isOutput=False)
    wt = nc.declare_dram_parameter("Wt", [V, D], F32, isOutput=False)
    encT = nc.declare_dram_parameter("encT", [2, 128, 2, RC], FP8, isOutput=False)
    enc_rows = nc.declare_dram_parameter("enc_rows", [RC, D], F32, isOutput=False)
    maskv = nc.declare_dram_parameter("maskv", [RC, 1], F32, isOutput=False)

    out_stats = nc.declare_dram_parameter("out_stats", [1, 8], F32, isOutput=True)
    out_pres = nc.declare_dram_parameter("out_pres", [V, 1], F32, isOutput=True)

    with tile.TileContext(nc) as tc, ExitStack() as ctx:
        const_p = ctx.enter_context(tc.tile_pool(name="const", bufs=1))
        small_p = ctx.enter_context(tc.tile_pool(name="small", bufs=4))
        cols_p = ctx.enter_context(tc.tile_pool(name="cols", bufs=1))
        stage0_p = ctx.enter_context(tc.tile_pool(name="stage0", bufs=2))
        scr_p = ctx.enter_context(tc.tile_pool(name="scr", bufs=2))
        psum_p = ctx.enter_context(tc.tile_pool(name="ps", bufs=4, space="PSUM"))

        # ---------------- constants / persistent tiles ----------------
        ident = const_p.tile([128, 128], F32)
        make_identity(nc, ident[:])
        ones_t = const_p.tile([128, 1], F32)
        nc.vector.memset(ones_t[:], 1.0)
        margin_t = const_p.tile([128, 1], F32)
        nc.vector.memset(margin_t[:], MARGIN)
        offs_t = const_p.tile([128, NVC], F32)
        for h in range(NVC):
            nc.vector.memset(offs_t[:, h:h + 1], float(h * VC))
        smalls_sb = const_p.tile([128, 4 * EDIM], F32, name="smalls_sb")
        nc.sync.dma_start(smalls_sb[:], smalls[:])
        pj = [smalls_sb[:, 0:16], smalls_sb[:, 16:32], smalls_sb[0:64, 32:48]]
        csum_b = smalls_sb[:, 48:64]

        # All loads on the sync ring in strict consumption order (parallel
        # rings share HBM bandwidth fairly and delay the critical prefix).
        st_sb = []
        for kc, (k0, k1) in enumerate([(0, 128), (128, 256), (256, 320)]):
            t_ = const_p.tile([k1 - k0, RC], F32, name=f"stackT_{kc}")
            st_sb.append(t_)

        ek_sb = const_p.tile([128, V], BF16)
        et_sb = [const_p.tile([128, 2, RC], FP8, name=f"encT_{kc}")
                 for kc in range(2)]
        w_sb = [const_p.tile([128, 2, V], FP8, name=f"w_sb_{kc}")
                for kc in range(2)]
        er_all = const_p.tile([128, RT * D], F32, name="er_all")
        gt_dump = const_p.tile([128, 2 * VC], BF16, name="gt_dump")
        act_dump = const_p.tile([128, D], BF16, name="act_dump")

        def emit_ek_rest():
            for h in range(1, NVC):
                nc.sync.dma_start(ek_sb[:, h * VC:(h + 1) * VC],
                                  ek[:, h * VC:(h + 1) * VC])
            _er = enc_rows[:]
            nc.sync.dma_start(
                er_all[:],
                bass.AP(tensor=_er.tensor, offset=_er.offset,
                        ap=[[D, 128], [128 * D, RT], [1, D]]))

        def emit_late_loads():
            for kc in range(2):
                nc.sync.dma_start(w_sb[kc][:, :, 0:VC], w[kc, :, :, 0:VC])
            for kc in range(2):
                nc.sync.dma_start(et_sb[kc][:], encT[kc, :, :, :])
            for h in range(1, NVC):
                for kc in range(2):
                    nc.sync.dma_start(
                        w_sb[kc][:, :, h * VC:(h + 1) * VC],
                        w[kc, :, :, h * VC:(h + 1) * VC])

        xk_bufs = []
        for i_ in range(RT):
            xkt = const_p.tile([128, 128], BF16, name=f"xk_{i_}")
            nc.vector.memset(xkt[:], 0.0)
            xk_bufs.append(xkt)

        s_cols = cols_p.tile([128, RT, NVC], F32)
        c_cols = cols_p.tile([128, RT, NVC], F32)
        nc.vector.memset(c_cols[:], 0.0)
        ltgt_cols = cols_p.tile([128, RT], F32)
        ethr_cols = cols_p.tile([128, RT], F32)
        nethr_cols = cols_p.tile([128, RT], F32)
        mask_cols = cols_p.tile([128, RT], F32)
        red_cols = cols_p.tile([128, 2 * RT], F32)

        _mv = maskv[:]
        nc.sync.dma_start(mask_cols[:],
                          bass.AP(tensor=_mv.tensor, offset=_mv.offset,
                                  ap=[[1, 128], [128, RT]]))

        # ---------------- stage 0: Kahan x tiles ------------------------
        c2 = min(256, RC)
        for kc, (k0, k1) in enumerate([(0, 128), (128, 256), (256, 320)]):
            nc.sync.dma_start(st_sb[kc][:, 0:c2], stackT[k0:k1, 0:c2])
        stk_tiles = [stage0_p.tile([128, SF], F32, name=f"stk_{rt}", bufs=RT)
                     for rt in range(RT)]
        for rt in range(min(2, RT)):
            nc.sync.dma_start(stk_tiles[rt][:],
                              stack_rows[rt * 128:(rt + 1) * 128, :])
        nc.sync.dma_start(ek_sb[:, 0:VC], ek[:, 0:VC])
        if c2 < RC:
            for kc, (k0, k1) in enumerate([(0, 128), (128, 256), (256, 320)]):
                nc.sync.dma_start(st_sb[kc][:, c2:RC], stackT[k0:k1, c2:RC])

        def emit_stage0(rt):
            c0 = rt * 128
            msum = small_p.tile([128, 1], F32, name="msum")
            nc.vector.reduce_sum(msum[:], stk_tiles[rt][:],
                                 axis=mybir.AxisListType.X)
            msum = msum[:]

            psq = psum_p.tile([128, EDIM], F32, name="psq", tag="big")
            for kc in range(3):
                nc.tensor.matmul(psq[:], st_sb[kc][:, c0:c0 + 128], pj[kc],
                                 start=(kc == 0), stop=(kc == 2))
            mu_c = small_p.tile([128, EDIM], F32, name="mu_c")
            nc.vector.tensor_scalar(mu_c[:], csum_b, msum, None, ALU.mult)
            x_t = small_p.tile([128, EDIM], F32, name="x_t")
            nc.vector.tensor_tensor(out=x_t[:], in0=psq[:], in1=mu_c[:],
                                    op=ALU.subtract)

            pst = psum_p.tile([16, 128], F32, name="pst", tag="big")
            nc.tensor.transpose(pst[:], x_t[:], ident[:])

            xk = xk_bufs[rt]
            xh_f = small_p.tile([16, 128], F32, name="xh_f")
            nc.vector.tensor_copy(xk[0:16, :], pst[:])           # hi (cast)
            nc.vector.tensor_copy(xh_f[:], xk[0:16, :])          # back to f32
            nc.vector.tensor_tensor(out=xh_f[:], in0=pst[:], in1=xh_f[:],
                                    op=ALU.subtract)             # residual
            nc.vector.tensor_copy(xk[32:48, :], xh_f[:])         # lo (cast)
            nc.vector.tensor_copy(xk[64:80, :], xk[0:16, :])
            nc.vector.tensor_copy(xk[96:112, :], xk[32:48, :])

        # ---------------- pipelined main loop ---------------------------
        i8_tiles = [None] * RT

        mi_tiles = [None] * RT

        def emit_a_chunk(rt, h):
            if h == 0:
                # 4 chunk scans write straight into [m8_h | i8_h] slots
                mi_tiles[rt] = (
                    small_p.tile([128, NVC * 8], F32, name=f"mi_{rt}", bufs=RT),
                    small_p.tile([128, NVC * 8], U32, name=f"ix_{rt}", bufs=RT))
            mi, ix = mi_tiles[rt]
            psd = psum_p.tile([128, VC], F32, name="psd", tag="big")
            for j in range(VC // 512):
                nc.tensor.matmul(
                    psd[:, j * 512:(j + 1) * 512], xk_bufs[rt][:],
                    ek_sb[:, h * VC + j * 512:h * VC + (j + 1) * 512],
                    start=True, stop=True)
            nc.vector.max(mi[:, h * 8:(h + 1) * 8], psd[:])
            nc.vector.max_index(ix[:, h * 8:(h + 1) * 8],
                                mi[:, h * 8:(h + 1) * 8], psd[:])

        def emit_a_scan(rt):
            mi, ix = mi_tiles[rt]
            cm = mi[:, 0:NVC * 8:8]                  # chunk maxima [128, 4]
            # global max over the 4 chunk maxima, then select its index
            gm = small_p.tile([128, 1], F32, name="gm")
            nc.vector.tensor_reduce(op=ALU.max, out=gm[:], in_=cm,
                                    axis=mybir.AxisListType.X)
            eq = small_p.tile([128, NVC], F32, name="eq")
            nc.vector.tensor_scalar(eq[:], cm, gm[:], None, ALU.is_equal)
            ii = small_p.tile([128, NVC], F32, name="ii")
            nc.vector.tensor_copy(ii[:], ix[:, 0:NVC * 8:8])    # u32 -> f32
            nc.vector.tensor_tensor(out=ii[:], in0=ii[:], in1=offs_t[:],
                                    op=ALU.add)
            nc.vector.tensor_tensor(out=ii[:], in0=ii[:], in1=eq[:],
                                    op=ALU.mult)
            idxf = small_p.tile([128, 1], F32, name="idxf")
            nc.vector.reduce_sum(idxf[:], ii[:], axis=mybir.AxisListType.X)
            # pad rows (all-zero dots) match every chunk and sum all four
            # chunk offsets -> 12288 > V-1; bounds_check drops them in both
            # the presence scatter and the Wt gather (host forces pres[0]).
            i8 = small_p.tile([128, 1], U32, name=f"i8_{rt}", bufs=RT)
            nc.vector.tensor_copy(i8[:], idxf[:])               # f32 -> u32
            i8_tiles[rt] = i8
            nc.gpsimd.indirect_dma_start(
                out=out_pres[:], out_offset=IndirectOffsetOnAxis(
                    ap=i8[:, :1], axis=0),
                in_=ones_t[:, :], in_offset=None,
                bounds_check=V - 1, oob_is_err=False)

        def emit_prep(rt):
            g_t = scr_p.tile([128, D], F32, name="g_t")
            nc.gpsimd.indirect_dma_start(
                out=g_t[:], out_offset=None, in_=wt[:],
                in_offset=IndirectOffsetOnAxis(ap=i8_tiles[rt][:, 0:1], axis=0),
                bounds_check=V - 1, oob_is_err=False)
            prod = scr_p.tile([128, D], F32, name="prod")
            nc.vector.tensor_tensor(out=prod[:], in0=er_all[:, rt * D:(rt + 1) * D],
                                    in1=g_t[:], op=ALU.mult)
            nc.scalar.activation(act_dump[:], prod[:], AF.Copy,
                                 accum_out=ltgt_cols[:, rt:rt + 1])
            # ethr = exp(ltgt + margin); nethr = -ethr: relu-count bias
            # (stage B's exp is bias-free so it never waits on this chain)
            nc.scalar.activation(ethr_cols[:, rt:rt + 1],
                                 ltgt_cols[:, rt:rt + 1], AF.Exp,
                                 bias=margin_t[:])
            nc.vector.tensor_scalar(nethr_cols[:, rt:rt + 1],
                                    ethr_cols[:, rt:rt + 1], -1.0, None,
                                    ALU.mult)

        exp_rows = [None]

        def emit_b_chunk(rt, h):
            psl = psum_p.tile([128, VC], F32, name="psl", tag="big")
            for kc in range(2):
                for j in range(VC // 512):
                    nc.tensor.matmul(
                        psl[:, j * 512:(j + 1) * 512],
                        et_sb[kc][:, :, rt * 128:(rt + 1) * 128],
                        w_sb[kc][:, :, h * VC + j * 512:h * VC + (j + 1) * 512],
                        start=(kc == 0), stop=(kc == 1),
                        perf_mode=mybir.MatmulPerfMode.DoubleRow)
            if h % 2 == 0:
                exp_rows[0] = scr_p.tile([128, 2 * VC], BF16, name="exp_row")
            exp_row = exp_rows[0]
            seg = exp_row[:, (h % 2) * VC:(h % 2 + 1) * VC]
            nc.scalar.activation(seg, psl[:], AF.Exp, scale=1.0 / (SC_W * SC_E),
                                 accum_out=s_cols[:, rt, h:h + 1])
            # corr count: sum_v relu(exp(l) - exp(ltgt+margin)) is EXACTLY 0
            # for correct rows (every term negative -> relu gives exact 0),
            # >= ~0.008 otherwise.  The last two row tiles count on DVE
            # (is_gt) instead: ACT is the tail bottleneck there.
            if rt >= RT - 1:
                nc.vector.tensor_scalar(gt_dump[:, 0:VC], seg,
                                        ethr_cols[:, rt:rt + 1], None,
                                        ALU.is_gt, ALU.add,
                                        accum_out=c_cols[:, rt, h:h + 1])
            elif h % 2 == 1:
                nc.scalar.activation(gt_dump[:], exp_row[:], AF.Relu,
                                     bias=nethr_cols[:, rt:rt + 1],
                                     accum_out=c_cols[:, rt,
                                                      h // 2:h // 2 + 1])

        emit_stage0(0)
        if RT > 1:
            emit_stage0(1)
        emit_ek_rest()
        for h in range(NVC):
            emit_a_chunk(0, h)
        emit_a_scan(0)
        emit_prep(0)
        for rt in range(2, RT):
            nc.sync.dma_start(stk_tiles[rt][:],
                              stack_rows[rt * 128:(rt + 1) * 128, :])
        emit_late_loads()
        for rt in range(RT):
            if 2 <= rt + 1 < RT:
                emit_stage0(rt + 1)
            for h in range(NVC):
                if rt + 1 < RT:
                    emit_a_chunk(rt + 1, h)
                emit_b_chunk(rt, h)
            if rt + 1 < RT:
                emit_a_scan(rt + 1)
                emit_prep(rt + 1)

        # ---------------- finalize --------------------------------------
        s_sum = small_p.tile([128, RT], F32, name="s_sum")
        nc.vector.reduce_sum(s_sum[:], s_cols[:], axis=mybir.AxisListType.X)
        lnS = small_p.tile([128, RT], F32, name="lnS")
        nc.scalar.activation(lnS[:], s_sum[:], AF.Ln)
        nll = small_p.tile([128, RT], F32, name="nll")
        nc.vector.tensor_tensor(out=nll[:], in0=lnS[:], in1=ltgt_cols[:],
                                op=ALU.subtract)
        nc.vector.tensor_tensor(out=red_cols[:, 0:RT], in0=nll[:],
                                in1=mask_cols[:], op=ALU.mult)
        cnt = small_p.tile([128, RT], F32, name="cnt")
        nc.vector.reduce_sum(cnt[:], c_cols[:], axis=mybir.AxisListType.X)
        corr = small_p.tile([128, RT], F32, name="corr")
        nc.vector.tensor_scalar(corr[:], cnt[:], 1e-3, None, ALU.is_lt)
        nc.vector.tensor_tensor(out=red_cols[:, RT:2 * RT], in0=corr[:],
                                in1=mask_cols[:], op=ALU.mult)

        psr = psum_p.tile([1, 2 * RT], F32, name="psr", tag="big")
        nc.tensor.matmul(psr[:], ones_t[:], red_cols[:], start=True, stop=True)
        fin = small_p.tile([1, 8], F32, name="fin")
        nc.vector.reduce_sum(fin[:, 0:1], psr[0:1, 0:RT],
                             axis=mybir.AxisListType.X)
        nc.vector.reduce_sum(fin[:, 1:2], psr[0:1, RT:2 * RT],
                             axis=mybir.AxisListType.X)
        nc.vector.memset(fin[:, 2:8], 0.0)
        nc.sync.dma_start(out_stats[:], fin[:])

    nc.compile()
    _NC_CACHE[RT] = nc
    return nc


def _pack_dr(a):
    """Pack [K, N] (K=512) into DoubleRow fp8 layout [2, 128, 2, N]:
    out[kc, p, i, n] = a[kc*256 + i*128 + p, n]."""
    K, Nn = a.shape
    out = np.asarray(a, np.float32).reshape(2, 2, 128, Nn).transpose(0, 2, 1, 3)
    return np.ascontiguousarray(out.astype(ml_dtypes.float8_e4m3))


def _plan(lengths):
    L = (np.asarray(lengths).astype(np.int64) // STRIDE)
    bs, ts = [], []
    for b in range(B):
        nb = max(0, int(L[b]) - 1)
        bs.append(np.full(nb, b, dtype=np.int64))
        ts.append(np.arange(1, nb + 1, dtype=np.int64))
    bs = np.concatenate(bs)
    ts = np.concatenate(ts)
    M = len(bs)
    percore = -(-M // NCORES)
    RT = max(1, -(-percore // 128))
    return bs, ts, M, percore, RT


def _prep_core_inputs(inputs, core, plan):
    bs, ts, M, percore, RT = plan
    RC = RT * 128
    feats = np.asarray(inputs["feats"], dtype=np.float32)
    enc = np.asarray(inputs["encoder_out"], dtype=np.float32)

    lo = core * percore
    hi = min(M, lo + percore)
    cb, ct = bs[lo:hi], ts[lo:hi]
    nv = len(cb)

    st_all = feats.reshape(B, N, SF)
    stack_rows = np.zeros((RC, SF), np.float32)
    stack_rows[:nv] = st_all[cb, ct]
    enc_rows = np.zeros((RC, D), np.float32)
    enc_rows[:nv] = enc[cb, ct - 1]
    maskv = np.zeros((RC, 1), np.float32)
    maskv[:nv] = 1.0

    return {
        "stack_rows": stack_rows,
        "stackT": np.ascontiguousarray(stack_rows.T),
        "encT": _pack_dr(enc_rows.T * SC_E),
        "enc_rows": enc_rows,
        "maskv": maskv,
    }


def _prep_shared_inputs(inputs):
    proj = np.asarray(inputs["projection"], dtype=np.float32)
    emb = np.asarray(inputs["embeddings"], dtype=np.float32)
    top = np.asarray(inputs["top_n_out"], dtype=np.float32)

    csums = (proj.sum(0, keepdims=True) / SF).astype(np.float32)  # [1, 16]
    smalls = np.zeros((128, 4 * EDIM), np.float32)
    smalls[:, 0:16] = proj[0:128]
    smalls[:, 16:32] = proj[128:256]
    smalls[0:64, 32:48] = proj[256:320]
    smalls[:, 48:64] = csums  # broadcast across partitions

    Et = np.ascontiguousarray(emb[:, 0, :].T, dtype=np.float32)  # [16, V]
    Eh = Et.astype(ml_dtypes.bfloat16).astype(np.float32)
    El = (Et - Eh).astype(ml_dtypes.bfloat16).astype(np.float32)
    Z = np.zeros_like(Eh)
    # row pairing with x tile [xh;0;xl;0;xh;0;xl;0]: hh + lh + hl + ll
    Ek = np.concatenate(
        [Eh, Z, Eh, Z, El, Z, El, Z], axis=0).astype(ml_dtypes.bfloat16)

    W = np.ascontiguousarray(top[0, 0], dtype=np.float32)        # [D, V]
    return {
        "smalls": smalls,
        "Ek": np.ascontiguousarray(Ek),
        "W": _pack_dr(W * SC_W),
        "Wt": np.ascontiguousarray(W.T),
    }


def _combine(results, inputs):
    lengths = np.asarray(inputs["feats_lengths"]).astype(np.int64)
    L = lengths // STRIDE
    num_codes = float(np.maximum(L - 1, 0).sum())

    nll_sum = 0.0
    corr_sum = 0.0
    pres = np.zeros(V, dtype=bool)
    for r in results:
        st = np.asarray(r["out_stats"]).reshape(-1)
        nll_sum += float(st[0])
        corr_sum += float(st[1])
        pres |= np.asarray(r["out_pres"]).reshape(-1) > 0.0
    pres[0] = True  # unmasked rows always scatter index 0 in the reference

    loss = np.float32(nll_sum / num_codes)
    acc = np.float32(corr_sum / num_codes)
    uniq = np.float32(pres.sum())
    return np.array([loss, acc, np.float32(num_codes), uniq], dtype=np.float32)


def _run(inputs, trace=False):
    from concourse.bass_utils import run_bass_kernel_spmd
    plan = _plan(inputs["feats_lengths"])
    nc = _build_program(plan[4])
    shared = _prep_shared_inputs(inputs)
    in_maps = []
    for core in range(NCORES):
        m = dict(shared)
        m.update(_prep_core_inputs(inputs, core, plan))
        in_maps.append(m)
    res = run_bass_kernel_spmd(nc, in_maps, core_ids=list(range(NCORES)),
                               trace=trace)
    out = _combine(res.results, inputs)
    return out, res


def _run_sim(inputs, core=0):
    """Single-core simulator run (correctness debugging)."""
    from concourse.bass_interp import CoreSim
    plan = _plan(inputs["feats_lengths"])
    nc = _build_program(plan[4])
    m = dict(_prep_shared_inputs(inputs))
    m.update(_prep_core_inputs(inputs, core, plan))
    sim = CoreSim(nc)
    for k, v in m.items():
        sim.tensor(k)[:] = v
    sim.simulate()
    return {k: np.array(sim.tensor(k)) for k in ("out_stats", "out_pres")}


def kernel(**inputs) -> np.ndarray:
    out, _ = _run(inputs, trace=False)
    return out
